# revision 13
# baseline (speedup 1.0000x reference)
"""Multi-head attention (B=2, S=2048, D=1024, H=16) on 8 Trainium2 cores.

Sharding: core c handles (batch b = c//4, head-group g = c%4 of 4 heads).
Megatron-style: W_q/k/v rows (output dims) column-sharded per head-group;
W_o columns row-sharded; the all-reduce over head-groups happens on the host
at gather time (sum of 4 partial projections per batch), where b_o is added.

Cost-model facts this kernel is shaped around (bass_rust cost model):
  - matmul cost = out free-size x pe_cycle (K-INDEPENDENT); bf16 1 cyc/row.
  - PE p-state: >3us of continuous busy -> 2.4 GHz; an idle gap resets the
    ramp to 1.2 GHz.  Sub-~0.7us gaps are tolerated; multi-us gaps demote.
  - ACT runs at a fixed 1.2 GHz; exp of the 4x2048x2048 score matrix
    (~571ns per [128,512] tile, 256 tiles) is the kernel bottleneck
    (~146us).  Everything else is arranged to hide under that stream.

Device layout (per core):
  Phase A: v projection (seq on partitions), v stored [128, 16 s-chunks,
  4 heads, 65] with a ones column per head (row-sum accumulator in A@V).
  Phase B: q/k projections transposed: q^T,k^T [128, 2, S] (k first).
  Phase C: per (pr, ih, hh) pass: scores^T [j, i] tiles [128, 512] via K=64
  matmuls, exp on ScalarE straight out of PSUM (scale=1/8, no max
  subtraction: scores ~ N(0,1)), then A@V in [i-part, d-free] layout:
  out[i, d] += ex^T[i, j-chunk] @ v[j-chunk, d] -- 65-cycle matmuls (the
  cost model charges out-free-size only), accumulating psA [128 i, 8 ic,
  65(+pad)] over 16 j-chunks.  Row-sums land in column 64 per PARTITION, so
  normalization is two tiny DVE ops per i-chunk (reciprocal + per-partition
  tensor_scalar multiply) -- no gpsimd broadcast, no partition-shift DMAs.
  Phase T: normalized ctx [i, d] tiles are transposed back to [d-pair, i]
  with PE transpose matmuls against a DMA'd identity (128 cycles each).
  Phase D: output projection with head-pair-stacked ctx2 [128, 2, S] and
  W_o^T [128, 2, 1024]: K=128 matmuls.
  T and D for early s-tiles are interleaved into later phase-C passes as PE
  filler (PE is ~45% idle under the ACT exp stream); only st2/st3 trail.
"""

import numpy as np
from collections import deque
from contextlib import ExitStack

import concourse.bass as bass
import concourse.bacc as bacc
import concourse.tile as tile
from concourse import mybir
from concourse.bass_utils import run_bass_kernel_spmd

F32 = mybir.dt.float32
BF16 = mybir.dt.bfloat16
AF = mybir.ActivationFunctionType

B, S, D = 2, 2048, 1024
H, DH = 16, 64
NCORES = 8
LOC = D // 4          # 256 local dims per head-group
SCALE = 1.0 / np.sqrt(DH)

_CACHED_NC = None


def build_nc():
    nc = bacc.Bacc("TRN2", target_bir_lowering=False, debug=False)

    qt = nc.dram_tensor("qt", [D, S], BF16, kind="ExternalInput").ap()
    kt = nc.dram_tensor("kt", [D, S], BF16, kind="ExternalInput").ap()
    vt = nc.dram_tensor("vt", [D, S], BF16, kind="ExternalInput").ap()
    wqt = nc.dram_tensor("wqt", [D, LOC], BF16, kind="ExternalInput").ap()
    wkt = nc.dram_tensor("wkt", [D, LOC], BF16, kind="ExternalInput").ap()
    wvt = nc.dram_tensor("wvt", [D, LOC], BF16, kind="ExternalInput").ap()
    wot = nc.dram_tensor("wot", [128, 2, D], BF16, kind="ExternalInput").ap()
    bq = nc.dram_tensor("bq", [128, 2], F32, kind="ExternalInput").ap()
    bk = nc.dram_tensor("bk", [128, 2], F32, kind="ExternalInput").ap()
    bv = nc.dram_tensor("bv", [128, LOC], F32, kind="ExternalInput").ap()
    ident = nc.dram_tensor("ident", [128, 128], BF16, kind="ExternalInput").ap()
    outp = nc.dram_tensor("outp", [D, S], BF16, kind="ExternalOutput").ap()

    with tile.TileContext(nc) as tc:
        with ExitStack() as ctx:
            wsb = ctx.enter_context(tc.tile_pool(name="wsb", bufs=1))
            big = ctx.enter_context(tc.tile_pool(name="big", bufs=1))

            # persistent SBUF state
            qt_sb = big.tile([128, 2, S], BF16, name="qt_sb")
            kt_sb = big.tile([128, 2, S], BF16, name="kt_sb")
            v_sb = big.tile([128, 16, 4, 66], BF16, name="v_sb")
            # normalized ctx in [i-part, d-free] layout: [i, pr, icg, dpair]
            ctxn = big.tile([128, 2, 16, 128], BF16, name="ctxn")
            # head-pair-stacked context (transposed): [dpair, pr, i]
            ctx2 = big.tile([128, 2, S], BF16, name="ctx2")

            wq_sb = wsb.tile([128, 8, LOC], BF16, name="wq_sb")
            wk_sb = wsb.tile([128, 8, LOC], BF16, name="wk_sb")
            wv_sb = wsb.tile([128, 8, LOC], BF16, name="wv_sb")
            wo_sb = wsb.tile([128, 2, D], BF16, name="wo_sb")
            id_sb = wsb.tile([128, 128], BF16, name="id_sb")
            bq_sb = wsb.tile([128, 2], F32, name="bq_sb")
            bk_sb = wsb.tile([128, 2], F32, name="bk_sb")
            bv_sb = wsb.tile([128, LOC], F32, name="bv_sb")
            wup = wsb.tile([64, 128], BF16, name="wup")

            nc.gpsimd.memset(wup, 0.0)
            # ones column of v (accumulates softmax row-sums in A@V)
            nc.gpsimd.memset(v_sb[:, :, :, DH : DH + 1], 1.0)

            with ExitStack() as stage_ctx:
                stage = stage_ctx.enter_context(
                    tc.tile_pool(name="stage", bufs=1)
                )
                # s-quarter staging: each quarter holds ALL 1024 d-rows for
                # 512 s-columns, so phase A/B s-tiles unblock progressively
                vt_st = stage.tile([128, 8, S], BF16, name="vt_st")
                qt_st = stage.tile([128, 8, S], BF16, name="qt_st")
                kt_st = stage.tile([128, 8, S], BF16, name="kt_st")

                def _sq(x, x_st, q, eng):
                    eng.dma_start(
                        out=x_st[:, :, q * 512 : (q + 1) * 512],
                        in_=x[:, q * 512 : (q + 1) * 512].rearrange(
                            "(a p) s -> p a s", p=128
                        ),
                    )

                # 3-queue cohort staging in consumption order: wk + kt
                # first (phase B-k), then wv + vt (phase A), then wq + qt
                nc.gpsimd.dma_start(
                    out=wk_sb, in_=wkt.rearrange("(a p) r -> p a r", p=128)
                )
                _sq(kt, kt_st, 0, nc.sync)
                _sq(kt, kt_st, 1, nc.scalar)
                _sq(kt, kt_st, 2, nc.gpsimd)
                _sq(kt, kt_st, 3, nc.sync)
                nc.gpsimd.dma_start(
                    out=wv_sb, in_=wvt.rearrange("(a p) r -> p a r", p=128)
                )
                nc.gpsimd.dma_start(out=bv_sb, in_=bv)
                _sq(vt, vt_st, 0, nc.scalar)
                _sq(vt, vt_st, 1, nc.gpsimd)
                _sq(vt, vt_st, 2, nc.sync)
                nc.scalar.dma_start(
                    out=wq_sb, in_=wqt.rearrange("(a p) r -> p a r", p=128)
                )
                _sq(vt, vt_st, 3, nc.scalar)
                _sq(qt, qt_st, 0, nc.gpsimd)
                _sq(qt, qt_st, 1, nc.sync)
                _sq(qt, qt_st, 2, nc.scalar)
                _sq(qt, qt_st, 3, nc.gpsimd)
                nc.sync.dma_start(out=bq_sb, in_=bq)
                nc.sync.dma_start(out=bk_sb, in_=bk)
                nc.sync.dma_start(out=wo_sb, in_=wot)
                nc.scalar.dma_start(out=id_sb, in_=ident)

                # ---- Warmup: dummy matmuls ramp the PE while DMAs land ----
                with tc.tile_pool(name="wps", bufs=1, space="PSUM") as wps:
                    wp = wps.tile([64, 128], F32, name="wp")
                    for _ in range(48):
                        nc.tensor.matmul(
                            wp, lhsT=wup[:, 0:64], rhs=wup,
                            start=True, stop=True,
                        )

                # ---- Front compute: Bk (k lands first), then A (v), then
                # Bq st0/st1 -- one shared 8-bank PSUM pool ----
                bv3 = bv_sb.rearrange("p (h d) -> p h d", h=4)
                with tc.tile_pool(name="fps", bufs=8, space="PSUM") as fps:
                    def b_tile(x_st, w_sb, x_out, b_sb, st, pr, on_act):
                        ps = fps.tile([128, 512], F32, name="psb", tag="f")
                        for ds in range(8):
                            nc.tensor.matmul(
                                ps,
                                lhsT=w_sb[:, ds, pr * 128 : (pr + 1) * 128],
                                rhs=x_st[:, ds, st * 512 : (st + 1) * 512],
                                start=(ds == 0),
                                stop=(ds == 7),
                            )
                        if on_act:
                            nc.scalar.activation(
                                out=x_out[:, pr, st * 512 : (st + 1) * 512],
                                in_=ps, func=AF.Identity,
                                bias=b_sb[:, pr : pr + 1], scale=1.0,
                            )
                        else:
                            nc.vector.tensor_scalar_add(
                                x_out[:, pr, st * 512 : (st + 1) * 512],
                                ps, b_sb[:, pr : pr + 1],
                            )

                    for st in range(4):
                        for pr in range(2):
                            b_tile(kt_st, wk_sb, kt_sb, bk_sb, st, pr, True)
                    for sg in range(4):  # v projection, s-quarters
                        psv = [
                            fps.tile([128, 512], F32, name="psv", tag="f")[:, 0:LOC]
                            for _ in range(4)
                        ]
                        for ds in range(8):
                            for c in range(4):
                                s0 = sg * 512 + c * 128
                                nc.tensor.matmul(
                                    psv[c],
                                    lhsT=vt_st[:, ds, s0 : s0 + 128],
                                    rhs=wv_sb[:, ds, :],
                                    start=(ds == 0),
                                    stop=(ds == 7),
                                )
                        for c in range(4):
                            sc = sg * 4 + c
                            nc.vector.tensor_add(
                                v_sb[:, sc, :, 0:DH],
                                psv[c].rearrange("p (h d) -> p h d", h=4),
                                bv3,
                            )
                    for st in range(2):
                        for pr in range(2):
                            b_tile(qt_st, wq_sb, qt_sb, bq_sb, st, pr, False)

            # ---- Phase C: attention, with T/D interleaved as PE filler ----
            with (
                tc.tile_pool(name="expp", bufs=3) as expp,
                tc.tile_pool(name="qk2ps", bufs=2, space="PSUM") as qk2ps,
                tc.tile_pool(name="avps", bufs=1, space="PSUM") as avps,
                tc.tile_pool(name="tailp", bufs=1, space="PSUM") as tailp,
                tc.tile_pool(name="bproj", bufs=1, space="PSUM") as bproj,
                tc.tile_pool(name="nrm", bufs=4) as nrm,
                tc.tile_pool(name="pob", bufs=4) as pob,
            ):
                filler = deque()

                def emit_Bq_units(st, pr):
                    """Late q-projection (s-tile st, half pr) as filler:
                    four 2-matmul units + one copy unit."""
                    bt = bproj.tile([128, 512], F32, name="bt")
                    def mm_pair(d0):
                        def f():
                            for ds in (d0, d0 + 1):
                                nc.tensor.matmul(
                                    bt,
                                    lhsT=wq_sb[:, ds,
                                               pr * 128 : (pr + 1) * 128],
                                    rhs=qt_st[:, ds,
                                              st * 512 : (st + 1) * 512],
                                    start=(ds == 0),
                                    stop=(ds == 7),
                                )
                        return f
                    def bcopy():
                        nc.vector.tensor_scalar_add(
                            qt_sb[:, pr, st * 512 : (st + 1) * 512],
                            bt,
                            bq_sb[:, pr : pr + 1],
                        )
                    return [("D", mm_pair(d)) for d in (0, 2, 4, 6)] + [
                        ("T", bcopy)]

                for st_, pr_ in ((2, 0), (2, 1), (3, 0), (3, 1)):
                    filler.extend(emit_Bq_units(st_, pr_))

                def emit_T(pr, ih):
                    """Transpose the 8 normalized ctx i-chunks of (pr, ih)
                    into ctx2 [d-pair, i] via PE transpose + one DVE copy."""
                    pst = tailp.tile([128, 8, 128], BF16, name="pst",
                                     tag="tail")
                    def t_pair(ic):
                        def f():
                            for k in (0, 1):
                                nc.tensor.transpose(
                                    pst[:, ic + k, :],
                                    ctxn[:, pr, ih * 8 + ic + k, :],
                                    id_sb,
                                )
                        return f
                    def t_copy():
                        nc.vector.tensor_copy(
                            ctx2[:, pr, ih * 1024 : (ih + 1) * 1024], pst
                        )
                    return [("T", t_pair(ic)) for ic in (0, 2, 4, 6)] + [
                        ("T", t_copy)]

                def emit_D_unit(st, ec, ceng):
                    """One phase-D output chunk: [128, 512] over K=256."""
                    def f():
                        pp = tailp.tile([128, 512], F32, name="pp", tag="tail")
                        for hp in range(2):
                            nc.tensor.matmul(
                                pp,
                                lhsT=wo_sb[:, hp, ec * 128 : (ec + 1) * 128],
                                rhs=ctx2[:, hp, st * 512 : (st + 1) * 512],
                                start=(hp == 0),
                                stop=(hp == 1),
                            )
                        ob = pob.tile([128, 512], BF16, name="ob")
                        ceng[0](ob, pp)
                        ceng[1](
                            out=outp[ec * 128 : (ec + 1) * 128,
                                     st * 512 : (st + 1) * 512],
                            in_=ob,
                        )
                    return f

                dve_c = (lambda o, i: nc.vector.tensor_copy(o, i),
                         nc.sync.dma_start)
                dve_c2 = (lambda o, i: nc.vector.tensor_copy(o, i),
                          nc.gpsimd.dma_start)

                for blk, (pr, ih) in enumerate(
                    [(0, 0), (1, 0), (0, 1), (1, 1)]
                ):
                    for hh in range(2):
                        h = 2 * pr + hh
                        r0, r1 = hh * 64, (hh + 1) * 64
                        psA = avps.tile([128, 8, 128], F32, name="psA")

                        def emit_qk(jc):
                            t = qk2ps.tile([128, 1024], F32, name="psqk2")
                            for s in range(2):
                                i0 = ih * 1024 + s * 512
                                nc.tensor.matmul(
                                    t[:, s * 512 : (s + 1) * 512],
                                    lhsT=kt_sb[r0:r1, pr,
                                               jc * 128 : (jc + 1) * 128],
                                    rhs=qt_sb[r0:r1, pr, i0 : i0 + 512],
                                    start=True,
                                    stop=True,
                                )
                            return t

                        def emit_exp_av(t, jc):
                            ex = expp.tile([128, 1024], BF16, name="ex")
                            nc.scalar.activation(
                                out=ex, in_=t, func=AF.Exp, scale=SCALE,
                            )
                            for ic in range(8):
                                # one start/stop per PSUM bank (= 4 ics):
                                # start lazily zeroes the whole 2KB zero
                                # region, so only the bank's first matmul
                                # starts and only its last stops
                                nc.tensor.matmul(
                                    psA[:, ic, 0:65],
                                    lhsT=ex[:, ic * 128 : (ic + 1) * 128],
                                    rhs=v_sb[:, jc, h, 0:65],
                                    start=(jc == 0 and ic % 4 == 0),
                                    stop=(jc == 15 and ic % 4 == 3),
                                )

                        prev = emit_qk(0)
                        for jc in range(1, 16):
                            cur = emit_qk(jc)
                            emit_exp_av(prev, jc - 1)
                            if filler and (jc % 4 == 1 or filler[0][0] == "T"):
                                filler.popleft()[1]()
                            prev = cur
                        emit_exp_av(prev, 15)
                        if filler:
                            filler.popleft()[1]()

                        # pass normalization, fused: row-sums are
                        # per-partition scalars (column 64 of psA)
                        rinv8 = nrm.tile([128, 8], F32, name="rinv8")
                        nc.vector.reciprocal_approx_fast(
                            out=rinv8, in_=psA[:, :, 64:65].squeeze(-1)
                        )
                        nc.vector.tensor_mul(
                            ctxn[:, pr, ih * 8 : (ih + 1) * 8,
                                 hh * 64 : (hh + 1) * 64],
                            psA[:, :, 0:DH],
                            rinv8.unsqueeze(-1).broadcast_to([128, 8, DH]),
                        )

                    # enqueue filler now ready: transposes of this block;
                    # D s-tiles once both ih=0 blocks are transposed
                    if blk == 0:
                        filler.extend(emit_T(0, 0))
                    elif blk == 1:
                        filler.extend(emit_T(1, 0))
                        filler.extend(
                            ("D", emit_D_unit(0, ec, dve_c if ec % 2 else dve_c2))
                            for ec in range(8)
                        )
                    elif blk == 2:
                        filler.extend(emit_T(0, 1))
                        filler.extend(
                            ("D", emit_D_unit(1, ec, dve_c if ec % 2 else dve_c2))
                            for ec in range(8)
                        )

                # flush any unpopped filler before pools close
                while filler:
                    filler.popleft()[1]()

            # ---- Tail: transpose (1,1), phase D st2/st3 ----
            with (
                tc.tile_pool(name="trp", bufs=2, space="PSUM") as trp,
                tc.tile_pool(name="pps2", bufs=4, space="PSUM") as pps2,
                tc.tile_pool(name="pob2", bufs=6) as pob2,
            ):
                pst = trp.tile([128, 8, 128], BF16, name="pst2")
                for ic in range(8):
                    nc.tensor.transpose(
                        pst[:, ic, :], ctxn[:, 1, 8 + ic, :], id_sb
                    )
                nc.vector.tensor_copy(ctx2[:, 1, 1024:2048], pst)

                for st in (2, 3):
                    for ec in range(8):
                        pp = pps2.tile([128, 512], F32, name="pp2")
                        for hp in range(2):
                            nc.tensor.matmul(
                                pp,
                                lhsT=wo_sb[:, hp, ec * 128 : (ec + 1) * 128],
                                rhs=ctx2[:, hp, st * 512 : (st + 1) * 512],
                                start=(hp == 0),
                                stop=(hp == 1),
                            )
                        ob = pob2.tile([128, 512], BF16, name="ob2")
                        if ec % 2 == 0:
                            nc.vector.tensor_copy(ob, pp)
                        else:
                            nc.scalar.activation(out=ob, in_=pp, func=AF.Copy)
                        dq = (nc.sync, nc.scalar, nc.gpsimd)[ec % 3]
                        dq.dma_start(
                            out=outp[ec * 128 : (ec + 1) * 128,
                                     st * 512 : (st + 1) * 512],
                            in_=ob,
                        )

    nc.compile()
    return nc


def _get_nc():
    global _CACHED_NC
    if _CACHED_NC is None:
        _CACHED_NC = build_nc()
    return _CACHED_NC


def make_in_maps(Q, K, V, W_q, b_q, W_k, b_k, W_v, b_v, W_o):
    import ml_dtypes

    BF = ml_dtypes.bfloat16
    xt = {}
    for b in range(B):
        xt["q", b] = np.ascontiguousarray(np.asarray(Q[b], np.float32).T).astype(BF)
        xt["k", b] = np.ascontiguousarray(np.asarray(K[b], np.float32).T).astype(BF)
        xt["v", b] = np.ascontiguousarray(np.asarray(V[b], np.float32).T).astype(BF)
    eye = np.eye(128, dtype=BF)
    in_maps = []
    for c in range(NCORES):
        b, g = divmod(c, 4)
        L = slice(g * LOC, (g + 1) * LOC)
        wqt = np.ascontiguousarray(np.asarray(W_q, np.float32)[L, :].T).astype(BF)
        wkt = np.ascontiguousarray(np.asarray(W_k, np.float32)[L, :].T).astype(BF)
        wvt = np.ascontiguousarray(np.asarray(W_v, np.float32)[L, :].T).astype(BF)
        # head-pair-stacked W_o^T: wot[p, hp, e] = W_o[e, g*256 + hp*128 + p]
        wot = np.ascontiguousarray(
            np.asarray(W_o, np.float32)[:, L].T.reshape(2, 128, D)
            .transpose(1, 0, 2).astype(BF)
        )
        bqh = np.ascontiguousarray(np.asarray(b_q, np.float32)[L].reshape(2, 128).T)
        bkh = np.ascontiguousarray(np.asarray(b_k, np.float32)[L].reshape(2, 128).T)
        bvh = np.ascontiguousarray(
            np.broadcast_to(np.asarray(b_v, np.float32)[L], (128, LOC))
        )
        in_maps.append(
            dict(
                qt=xt["q", b], kt=xt["k", b], vt=xt["v", b],
                wqt=wqt, wkt=wkt, wvt=wvt, wot=wot,
                bq=bqh, bk=bkh, bv=bvh, ident=eye,
            )
        )
    return in_maps


def gather(results, b_o):
    out = np.zeros((B, S, D), dtype=np.float32)
    for c in range(NCORES):
        b = c // 4
        out[b] += np.asarray(results[c]["outp"], np.float32).T
    out += np.asarray(b_o, np.float32)
    return out


def kernel(Q, K, V, W_q, b_q, W_k, b_k, W_v, b_v, W_o, b_o):
    nc = _get_nc()
    in_maps = make_in_maps(Q, K, V, W_q, b_q, W_k, b_k, W_v, b_v, W_o)
    res = run_bass_kernel_spmd(nc, in_maps, core_ids=list(range(NCORES)))
    return gather(res.results, b_o)
